# revision 6
# baseline (speedup 1.0000x reference)
"""Multi-head attention block on 8 TRN2 NeuronCores.

Problem: x[2,2048,768] -> qkv proj -> 12-head attention -> out proj.
Sharding: 24 (batch, head) pairs across 8 cores; core c handles batch
c//4 and heads 3*(c%4)..3*(c%4)+2. Each core computes its heads'
Q,K,V, attention, and a partial output projection; the host sums the
four per-batch partials and adds the bias terms.

Design notes (v3):
  - All matmul operands bf16 (halves DMA/SBUF; no f32r free<256
    penalty). Output bf16 too; host sums partials in f32. ~5.6e-3 rel
    error vs the 2e-2 gate.
  - The exp pipeline is the attention bottleneck per chunk (ScalarE
    ~1.04us vs PE's 853ns per [128,1024] chunk), so PE has ~18us of
    slack across the 96 chunks. All non-attention PE work (QKV blocks
    2/3, the whole first half of the output projection) is emitted as
    filler between attention chunks so PE never idles and the HAM
    p-state stays at 2.4GHz.
  - PSUM: s tiles [128,1024] double-buffered (4 banks) + two AV
    accumulators single-buffered (2 banks) live for the whole
    attention phase (outer pool); the QKV-projection pool (2 banks)
    closes after the interleaved phase-1 work and the out-projection
    pool (2 banks) reuses its banks.
  - AV accumulators are released by cheap PSUM->SBUF raw copies; the
    broadcast/reciprocal/normalize chain runs off the critical path
    reading the raw SBUF copy (per-head base-partition-0 tiles: a
    TensorTensor with both inputs in SBUF needs equal base
    partitions).
  - K and Q weights fused into one [C, 384] stationary stream (3
    passes per x^T block instead of 4). Q tiles pack two heads per 128
    partitions with no zero rows; K^T tiles carry the zeros (data
    top/bottom alternating) since only one matmul operand needs zeros
    to annihilate the other's don't-care rows.
  - PE p-state warmup matmuls run during the DMA prologue.
"""

import os
import sys

for _p in ("/opt/trn_rl_repo", "/opt/pypackages"):
    if _p not in sys.path:
        sys.path.append(_p)

import numpy as np

B, N, C = 2, 2048, 768
H, D = 12, 64
HPC = 3                    # heads per core
J = HPC * D                # 192 per-core head-dim rows
NCORES = 8
NBLK = 1024                # query-block width (one exp per [128, NBLK])
NB = N // NBLK             # 2
MC = N // 128              # 16 key chunks
KC = C // 128              # 6 contraction chunks for projections
NWARM = 5

_cache = {}
LAST_RESULTS = None


def _build():
    import concourse.mybir as mybir
    import concourse.tile as tile
    from concourse import bacc

    f32 = mybir.dt.float32
    bf16 = mybir.dt.bfloat16
    Exp = mybir.ActivationFunctionType.Exp
    mult = mybir.AluOpType.mult
    add = mybir.AluOpType.add

    nc = bacc.Bacc("TRN2", target_bir_lowering=False, debug=False,
                   num_devices=NCORES)

    xt_d = nc.declare_dram_parameter("xt", [C, N], bf16, isOutput=False)
    # fused [K_h0|K_h1|K_h2|Q_h0|Q_h1|Q_h2] weight columns
    wkq_d = nc.declare_dram_parameter("wkq", [C, 2 * J], bf16,
                                      isOutput=False)
    wv_d = nc.declare_dram_parameter("wv", [C, J], bf16, isOutput=False)
    bq_d = nc.declare_dram_parameter("bq", [J, 1], f32, isOutput=False)
    ones_d = nc.declare_dram_parameter("ones", [128, HPC], bf16,
                                       isOutput=False)
    zeros_d = nc.declare_dram_parameter("zeros", [64, N], bf16,
                                        isOutput=False)
    # padded proj weights: rows 0:128 = heads 0,1; 128:192 = head 2;
    # 192:256 = zero (annihilates ah2[1]'s junk bottom half)
    wp_d = nc.declare_dram_parameter("wp", [2 * 128, C], bf16,
                                     isOutput=False)
    warm_d = nc.declare_dram_parameter("warm", [128, 512], bf16,
                                       isOutput=False)
    out_d = nc.declare_dram_parameter("out", [N, C], bf16, isOutput=True)

    with tile.TileContext(nc) as tc:
        with (
            tc.tile_pool(name="persist", bufs=1) as pp,
            tc.tile_pool(name="osb", bufs=4) as posb,
            tc.tile_pool(name="etile", bufs=4) as pe,
            tc.tile_pool(name="bcsb", bufs=2) as pbc,
        ):
            warm_t = pp.tile([128, 512], bf16, tag="warm_t", name="warm_t")
            wkq = [pp.tile([128, 2 * J], bf16, tag=f"wkq{k}",
                           name=f"wkq{k}") for k in range(KC)]
            xt = [pp.tile([128, N], bf16, tag=f"xt{k}", name=f"xt{k}")
                  for k in range(KC)]
            wv = [pp.tile([128, J], bf16, tag=f"wv{k}", name=f"wv{k}")
                  for k in range(KC)]
            bqt = [pp.tile([64, 1], f32, tag=f"bq{h}", name=f"bq{h}")
                   for h in range(HPC)]
            ones_t = pp.tile([128, HPC], bf16, tag="ones_t", name="ones_t")
            wp = [pp.tile([128, C], bf16, tag=f"wp{t}", name=f"wp{t}")
                  for t in range(2)]
            # K^T per head, zero rows alternating so the packed Q tiles
            # need none: kh0 data 0:64, kh1 data 64:128, kh2 data 0:64
            kh = [pp.tile([128, N], bf16, tag=f"kh{h}", name=f"kh{h}")
                  for h in range(HPC)]
            # Q^T packed: qA = (q0 top, q1 bottom); qB = (q2 top, junk)
            qA = pp.tile([128, N], bf16, tag="qA", name="qA")
            qB = pp.tile([128, N], bf16, tag="qB", name="qB")
            # V with a ones column per head: [128, 3*65]
            vx = [pp.tile([128, HPC * 65], bf16, tag=f"vx{m}",
                          name=f"vx{m}") for m in range(MC)]
            sums = [pp.tile([1, N], f32, tag=f"sums{h}", name=f"sums{h}")
                    for h in range(HPC)]
            # unnormalized attention outputs (release AV PSUM fast),
            # base partition 0 so tensor_mul's SBUF inputs stay aligned
            raw = [pp.tile([64, N], bf16, tag=f"raw{h}", name=f"raw{h}")
                   for h in range(HPC)]
            ah2 = [pp.tile([128, N], bf16, tag=f"ah2{t}", name=f"ah2{t}")
                   for t in range(2)]

            # ---- input DMA, ordered for earliest compute start ----
            # sync queue: warmup data, fused KQ weights paired with the
            # first x^T column block, then the second block
            nc.sync.dma_start(warm_t[:], warm_d[:, :])
            for k in range(KC):
                nc.sync.dma_start(wkq[k][:], wkq_d[128 * k:128 * (k + 1), :])
                nc.sync.dma_start(xt[k][:, 0:512],
                                  xt_d[128 * k:128 * (k + 1), 0:512])
            for k in range(KC):
                nc.sync.dma_start(xt[k][:, 512:1024],
                                  xt_d[128 * k:128 * (k + 1), 512:1024])
            # gpsimd queue: V weights, x^T blocks 2/3
            for k in range(KC):
                nc.gpsimd.dma_start(wv[k][:], wv_d[128 * k:128 * (k + 1), :])
            for nb4 in (2, 3):
                nsl = slice(512 * nb4, 512 * (nb4 + 1))
                for k in range(KC):
                    nc.gpsimd.dma_start(xt[k][:, nsl],
                                        xt_d[128 * k:128 * (k + 1), nsl])
            # scalar queue (ACT idle early): small tensors, zero fills
            for h in range(HPC):
                nc.scalar.dma_start(bqt[h][:], bq_d[64 * h:64 * (h + 1), :])
            nc.scalar.dma_start(ones_t[:], ones_d[:, :])
            for t in range(2):
                nc.scalar.dma_start(wp[t][:], wp_d[128 * t:128 * (t + 1), :])
            nc.scalar.dma_start(kh[0][64:128, :], zeros_d[:, :])
            nc.scalar.dma_start(kh[1][0:64, :], zeros_d[:, :])
            nc.scalar.dma_start(kh[2][64:128, :], zeros_d[:, :])
            nc.scalar.dma_start(qB[64:128, :], zeros_d[:, :])
            nc.scalar.dma_start(ah2[1][64:128, :], zeros_d[:, :])
            for m in range(MC):
                on = vx[m].rearrange("p (h e) -> p h e", e=65)[:, :, 64:65]
                nc.vector.tensor_copy(
                    on, ones_t[:, :].rearrange("p (h e) -> p h e", e=1))

            with tc.tile_pool(name="ps2", bufs=1, space="PSUM") as ps2:
                pend = []

                def av_flush():
                    avh, h, nb, m, e = pend.pop(0)
                    vsl = slice(65 * h, 65 * (h + 1))
                    for i in range(2):
                        nc.tensor.matmul(
                            avh[i][:], vx[m][:, vsl],
                            e[:, 512 * i:512 * (i + 1)],
                            start=(m == 0), stop=(m == MC - 1))
                    if m != MC - 1:
                        return
                    # pair complete: free the AV accumulators with plain
                    # copies (denominator row + raw data), then normalize
                    # off the PE-critical path
                    adst, r0 = ((ah2[0], 0) if h == 0 else
                                (ah2[0], 64) if h == 1 else
                                (ah2[1], 0))
                    nsl = slice(NBLK * nb, NBLK * (nb + 1))
                    for i in range(2):
                        hf = slice(NBLK * nb + 512 * i,
                                   NBLK * nb + 512 * (i + 1))
                        nc.vector.tensor_copy(sums[h][:, hf],
                                              avh[i][64:65, :])
                        nc.vector.tensor_copy(raw[h][:, hf],
                                              avh[i][0:64, :])
                    bcs = pbc.tile([64, NBLK], f32, tag="bcs", name="bcs")
                    nc.gpsimd.partition_broadcast(bcs[:], sums[h][:, nsl])
                    rec = pbc.tile([64, NBLK], f32, tag="rec", name="rec")
                    nc.vector.reciprocal_approx_fast(rec[:], bcs[:])
                    for i in range(2):
                        hf = slice(NBLK * nb + 512 * i,
                                   NBLK * nb + 512 * (i + 1))
                        nc.vector.tensor_mul(
                            adst[r0:r0 + 64, hf], raw[h][:, hf],
                            rec[:, 512 * i:512 * (i + 1)])

                def attn_pair(h, nb, fillers=()):
                    """One (head, query-block) pair; fillers[i] (if any)
                    is emitted as PE slack-absorber after chunk i."""
                    qt = qA if h < 2 else qB
                    avh = [ps2.tile([65, 512], f32, tag=f"av{i}", bufs=1,
                                    name=f"ps_av{i}") for i in range(2)]
                    for m in range(MC):
                        msl = slice(128 * m, 128 * (m + 1))
                        s = ps2.tile([128, NBLK], f32, tag="s", bufs=2,
                                     name="ps_s")
                        for i in range(2):
                            q0 = NBLK * nb + 512 * i
                            nc.tensor.matmul(
                                s[:, 512 * i:512 * (i + 1)],
                                kh[h][:, msl], qt[:, q0:q0 + 512])
                        e = pe.tile([128, NBLK], bf16, tag="e", name="e")
                        nc.scalar.activation(e[:], s[:], Exp)
                        pend.append((avh, h, nb, m, e))
                        if len(pend) > 2:
                            av_flush()
                        if m < len(fillers) and fillers[m] is not None:
                            fillers[m]()

                with tc.tile_pool(name="ps1", bufs=1, space="PSUM") as ps1:
                    # PE p-state warmup during the DMA prologue
                    for i in range(NWARM):
                        ps = ps1.tile([128, 512], f32, tag="qk", bufs=2,
                                      name=f"warm{i}")
                        nc.tensor.matmul(ps[:], warm_t[:, 0:128], warm_t[:])

                    def g_group(b, g):
                        # g0: K_h0,K_h1 / g1: K_h2,Q_h0 / g2: Q_h1,Q_h2
                        nsl = slice(512 * b, 512 * (b + 1))
                        ps = ps1.tile([128, 512], f32, tag="qk", bufs=2,
                                      name="ps_qk")
                        for k in range(KC):
                            nc.tensor.matmul(
                                ps[:], wkq[k][:, 128 * g:128 * (g + 1)],
                                xt[k][:, nsl],
                                start=(k == 0), stop=(k == KC - 1))
                        if g == 0:
                            nc.vector.tensor_copy(kh[0][0:64, nsl],
                                                  ps[0:64, :])
                            nc.vector.tensor_copy(kh[1][64:128, nsl],
                                                  ps[64:128, :])
                        elif g == 1:
                            nc.vector.tensor_copy(kh[2][0:64, nsl],
                                                  ps[0:64, :])
                            nc.vector.tensor_scalar(
                                qA[0:64, nsl], ps[64:128, :], 0.125,
                                bqt[0][:], mult, add)
                        else:
                            nc.vector.tensor_scalar(
                                qA[64:128, nsl], ps[0:64, :], 0.125,
                                bqt[1][:], mult, add)
                            nc.vector.tensor_scalar(
                                qB[0:64, nsl], ps[64:128, :], 0.125,
                                bqt[2][:], mult, add)

                    def v_chunk(m):
                        msl = slice(128 * m, 128 * (m + 1))
                        ps = ps1.tile([128, 512], f32, tag="qk", bufs=2,
                                      name="ps_v")
                        for k in range(KC):
                            nc.tensor.matmul(ps[:, 0:J], xt[k][:, msl],
                                             wv[k][:],
                                             start=(k == 0),
                                             stop=(k == KC - 1))
                        vdst = vx[m].rearrange("p (h e) -> p h e",
                                               e=65)[:, :, 0:64]
                        nc.vector.tensor_copy(
                            vdst,
                            ps[:, 0:J].rearrange("p (h e) -> p h e", e=64))

                    for b in (0, 1):
                        for g in range(3):
                            g_group(b, g)
                        for m in range(4 * b, 4 * b + 4):
                            v_chunk(m)
                    # first pair absorbs phase-1 blocks 2/3 as filler
                    attn_pair(0, 0, fillers=[
                        (lambda b=b, g=g: g_group(b, g))
                        for b in (2, 3) for g in range(3)
                    ][:3] + [
                        lambda m=m: v_chunk(m) for m in range(8, 12)
                    ] + [
                        (lambda g=g: g_group(3, g)) for g in range(3)
                    ] + [
                        lambda m=m: v_chunk(m) for m in range(12, 16)
                    ])

                # phase-1 PSUM released; out-projection pool reuses it
                with tc.tile_pool(name="ps4", bufs=1, space="PSUM") as ps4:

                    def proj_chunk(m):
                        msl = slice(128 * m, 128 * (m + 1))
                        psa = ps4.tile([128, 512], f32, tag="pjA", bufs=1,
                                       name="ps_pjA")
                        psb = ps4.tile([128, 256], f32, tag="pjB", bufs=1,
                                       name="ps_pjB")
                        for t in range(2):
                            nc.tensor.matmul(psa[:], ah2[t][:, msl],
                                             wp[t][:, 0:512],
                                             start=(t == 0), stop=(t == 1))
                        for t in range(2):
                            nc.tensor.matmul(psb[:], ah2[t][:, msl],
                                             wp[t][:, 512:768],
                                             start=(t == 0), stop=(t == 1))
                        o3 = posb.tile([128, C], bf16, tag="o3", name="o3")
                        nc.vector.tensor_copy(o3[:, 0:512], psa[:])
                        nc.vector.tensor_copy(o3[:, 512:768], psb[:])
                        nc.sync.dma_start(out_d[msl, :], o3[:])

                    attn_pair(1, 0)
                    attn_pair(2, 0)
                    # nb0 projection chunks ride the nb1 pairs' PE slack
                    # (held back a few chunks so pair 2's normalize and
                    # pair 5's late fillers don't collide)
                    attn_pair(0, 1, fillers=[None] * 4 + [
                        (lambda m=m: proj_chunk(m)) for m in range(0, 4)
                    ])
                    attn_pair(1, 1, fillers=[None] * 2 + [
                        (lambda m=m: proj_chunk(m)) for m in range(4, 8)
                    ])
                    attn_pair(2, 1)
                    while pend:
                        av_flush()
                    for m in range(8, 16):
                        proj_chunk(m)

    nc.compile()
    return nc


def kernel(x, w_qkv, b_qkv, w_proj, b_proj):
    import ml_dtypes

    from concourse.bass_utils import run_bass_kernel_spmd

    global LAST_RESULTS
    if "nc" not in _cache:
        _cache["nc"] = _build()
    nc = _cache["nc"]

    bf16 = ml_dtypes.bfloat16
    x = np.asarray(x, dtype=np.float32)
    w_qkv = np.asarray(w_qkv, dtype=np.float32)
    b_qkv = np.asarray(b_qkv, dtype=np.float32)
    w_proj = np.asarray(w_proj, dtype=np.float32)
    b_proj = np.asarray(b_proj, dtype=np.float32)

    in_maps = []
    for c in range(NCORES):
        b = c // 4
        h0 = HPC * (c % 4)
        qs = slice(64 * h0, 64 * (h0 + HPC))
        ks = slice(C + 64 * h0, C + 64 * (h0 + HPC))
        vs = slice(2 * C + 64 * h0, 2 * C + 64 * (h0 + HPC))
        wkq = np.concatenate([w_qkv[:, ks], w_qkv[:, qs]], axis=1)
        wp_pad = np.zeros((2 * 128, C), dtype=np.float32)
        wp_pad[0:128] = w_proj[64 * h0:64 * (h0 + 2), :]
        wp_pad[128:192] = w_proj[64 * (h0 + 2):64 * (h0 + 3), :]
        in_maps.append({
            "xt": np.ascontiguousarray(x[b].T).astype(bf16),
            "wkq": np.ascontiguousarray(wkq).astype(bf16),
            "wv": np.ascontiguousarray(w_qkv[:, vs]).astype(bf16),
            "bq": np.ascontiguousarray(
                (b_qkv[qs] * 0.125).reshape(J, 1)).astype(np.float32),
            "ones": np.ones((128, HPC), dtype=bf16),
            "zeros": np.zeros((64, N), dtype=bf16),
            "wp": wp_pad.astype(bf16),
            "warm": np.ones((128, 512), dtype=bf16),
        })

    res = run_bass_kernel_spmd(nc, in_maps, core_ids=list(range(NCORES)))
    LAST_RESULTS = res

    out = np.zeros((B, N, C), dtype=np.float32)
    for c in range(NCORES):
        out[c // 4] += np.asarray(res.results[c]["out"], dtype=np.float32)
    out += b_proj + b_qkv[2 * C:] @ w_proj
    return out
